# revision 12
# baseline (speedup 1.0000x reference)
"""ALiBi causal attention on 8 TRN2 NeuronCores.

Sharding: core c handles batch b = c//4 and 4 heads (slope-grouped
permutation PERM0[c%4]); attention is fully local per core. The output
projection is computed as per-core PARTIAL products (own 256 features x
WoT rows) chunked by 512-query windows; a per-window 4-way ReduceScatter
(replica groups = batch quads) sums the partials and scatters each core
its own 128-row tile, writing straight into the bf16 output tensor
(host casts to f32). All but the last ReduceScatter hide under
attention compute (A2A has ~22-26us fixed cost; RS scales with bytes).

Score matmul trick: scores^T[j,i] = (q/8 . k)[j,i] + slope*j - slope*i
in one K=70 matmul; kT/qT rows 64-69 carry 3-term bf16 decompositions
of slope*j and -slope*i paired with ones. V gets a ones column so PV
also emits the softmax denominator (row 64).

Engine assignment (balance measured on traces):
- tensor: matmuls only.
- scalar (ACT): exp only (~59us) + early input-DMA issue.
- vector (DVE): all psum->sbuf copies, softmax normalize, memsets.
- gpsimd: warmup barrier, causal-mask adds, collective triggers.
- sync: staging/broadcast DMA issue.

Other structure:
- a zero-byte warmup AllGather fires at t~0 so core-start skew is
  absorbed on the CC engine concurrently with the fill phase instead of
  inside the first ReduceScatter.
- QK of pair n+1 issues before PV of pair n so exp hides under TensorE.
- RS_k's trigger is issued mid ichk k+1 so its semaphore wait doesn't
  block later gpsimd work (mask adds) while staging drains.
- reciprocals batched per head-pair on partitions 0/32 of one tile.
- ALiBi windows: dropped keys have slope*dist >= ~22 (weight < e^-18
  of max even with +-3 score noise).
"""

import sys

import numpy as np

try:
    import concourse  # noqa: F401
except ImportError:  # pragma: no cover
    sys.path.insert(0, "/opt/trn_rl_repo")

import ml_dtypes
from concourse import bacc, mybir
import concourse.tile as tile
from concourse.bass_utils import run_bass_kernel_spmd

BF16 = mybir.dt.bfloat16
F32 = mybir.dt.float32

B, T, DM, H = 2, 2048, 1024, 16
D = DM // H            # 64 head dim
NCORES = 8
QUAD = 4               # cores per batch
HPC = 4                # heads per core
PB = 128               # partitions
IC = 512               # i-chunk (query cols per window)
JT = 128               # j-tile (key rows per score tile)
NTT = T // PB          # 16 token tiles
NDC = DM // PB         # 8 d_model chunks
FPC = HPC * D          # 256 features per core
TOUT = T // QUAD       # 512 output rows per core
NEG = -1.0e9

import os as _os
PSA = int(_os.environ.get("PSA", 2))
PSS = int(_os.environ.get("PSS", 2))   # score tiles are 2 banks each
PSV = int(_os.environ.get("PSV", 2))
EPB = int(_os.environ.get("EPB", 4))

WTILES = [int(v) for v in _os.environ.get("WT", "1,3,11,16").split(",")]
PERM0 = [[m, 4 + m, 11 - m, 15 - m] for m in range(QUAD)]

_cache = {}


def _build():
    nc = bacc.Bacc("TRN2", target_bir_lowering=False, debug=False,
                   num_devices=NCORES)

    xw_e = nc.dram_tensor("xw", [PB, NDC * T], BF16, kind="ExternalInput")
    wqkv_e = nc.dram_tensor("wqkv", [PB, NDC * 3 * FPC], BF16,
                            kind="ExternalInput")
    wo_e = nc.dram_tensor("wo", [PB, 2 * DM], BF16, kind="ExternalInput")
    mask_e = nc.dram_tensor("mask", [PB, PB], BF16, kind="ExternalInput")
    kaug_e = nc.dram_tensor("kaug", [6 * HPC, T], BF16, kind="ExternalInput")
    qaug_e = nc.dram_tensor("qaug", [6 * HPC, T], BF16, kind="ExternalInput")
    out_e = nc.dram_tensor("out", [TOUT, DM], BF16, kind="ExternalOutput")

    from contextlib import ExitStack
    with tile.TileContext(nc) as tc, ExitStack() as es:
            def pool(**kw):
                return es.enter_context(tc.tile_pool(**kw))
            xtp = pool(name="xt", bufs=NDC)        # xT chunks
            wtp = pool(name="wt", bufs=NDC)        # wqkv chunks
            wop = pool(name="wo", bufs=1)          # WoT own heads
            qkp = pool(name="qk", bufs=2 * HPC)    # qT/kT (70,T)
            vp = pool(name="vp", bufs=4 * NTT)     # v tiles (128,65)
            smp = pool(name="small", bufs=1)       # mask
            rcp = pool(name="rcp", bufs=4)         # recip rows
            bcp = pool(name="bcp", bufs=2)         # broadcast recip
            ep = pool(name="ep", bufs=EPB)         # exp tiles
            op = pool(name="op", bufs=4)           # oT tiles (2 per ichk)
            rop = pool(name="ro", bufs=4)          # staged bf16 proj rows
            psA = pool(name="psA", bufs=PSA, space="PSUM")  # proj/outproj
            psS = pool(name="psS", bufs=PSS, space="PSUM")  # score (2 bank)
            psV = pool(name="psV", bufs=PSV, space="PSUM")  # pv
            rsi = [pool(name=f"rsi{k}", bufs=1, space="DRAM")
                   for k in range(4)]
            rso = [pool(name=f"rso{k}", bufs=1, space="DRAM")
                   for k in range(4)]
            wup = pool(name="wup", bufs=1, space="DRAM")

            # ---- warmup barrier: absorb core-start skew on the CC
            # engine while the fill phase runs. Input is never written
            # (garbage data, pure barrier), output never read.
            wu_in = wup.tile([1, PB], BF16, tag="wui")
            wu_out = wup.tile([NCORES, 1, PB], BF16, tag="wuo")
            nc.gpsimd.collective_compute(
                "AllGather", mybir.AluOpType.bypass,
                replica_groups=[list(range(NCORES))],
                ins=[wu_in.opt()], outs=[wu_out.opt()])

            # ---- constants ----
            mask = smp.tile([PB, PB], BF16, tag="mask")
            nc.sync.dma_start(out=mask[:, :], in_=mask_e[:, :])
            nb8 = smp.tile([PB, 1], F32, tag="nb8")
            nc.vector.memset(nb8[:, :], -8.0)

            # ---- input DMAs. w chunks first (q/k proj contracts all of
            # them), then x token-halves in need order. DMA transfers
            # serialize through the 16-engine pool, so order == arrival.
            xT = [xtp.tile([PB, T], BF16, tag="xt", name=f"xT{dc}")
                  for dc in range(NDC)]
            wT = [wtp.tile([PB, 3 * FPC], BF16, tag="wt", name=f"wT{dc}")
                  for dc in range(NDC)]
            for dc in range(NDC):
                eng = nc.scalar if dc % 2 == 0 else nc.sync
                eng.dma_start(out=wT[dc][:, :],
                              in_=wqkv_e[:, dc * 3 * FPC:(dc + 1) * 3 * FPC])
            for dc in range(NDC):
                eng = nc.scalar if dc % 2 == 1 else nc.sync
                eng.dma_start(out=xT[dc][:, 0:T // 2],
                              in_=xw_e[:, dc * T:dc * T + T // 2])
            woT = wop.tile([PB, 2 * DM], BF16, tag="wo")
            nc.scalar.dma_start(out=woT[:, :], in_=wo_e[:, :])
            for dc in range(NDC):
                eng = nc.scalar if dc % 2 == 0 else nc.sync
                eng.dma_start(out=xT[dc][:, T // 2:T],
                              in_=xw_e[:, dc * T + T // 2:(dc + 1) * T])

            qTt = [qkp.tile([70, T], BF16, tag="qk", name=f"qT{l}")
                   for l in range(HPC)]
            kTt = [qkp.tile([70, T], BF16, tag="qk", name=f"kT{l}")
                   for l in range(HPC)]
            for l in range(HPC):
                nc.sync.dma_start(out=kTt[l][64:70, :],
                                  in_=kaug_e[6 * l:6 * l + 6, :])
                nc.sync.dma_start(out=qTt[l][64:70, :],
                                  in_=qaug_e[6 * l:6 * l + 6, :])

            vt = {}
            for l in range(HPC):
                vt[l] = [vp.tile([PB, D + 1], BF16, tag="vp",
                                 name=f"v{l}_{tt}")
                         for tt in range(NTT)]

            # ---- projections for one 512-token chunk (all 4 heads) ----
            def proj(tch):
                with nc.named_scope(f"proj{tch}", notify=True):
                    for wi, dest, scl in ((0, qTt, 0.125), (1, kTt, None)):
                        for fb in range(2):
                            pp = psA.tile([PB, IC], F32, tag="pp",
                                          name=f"qk{wi}{fb}{tch}")
                            for dc in range(NDC):
                                nc.tensor.matmul(
                                    pp[:, :],
                                    wT[dc][:, wi * FPC + fb * PB:
                                           wi * FPC + (fb + 1) * PB],
                                    xT[dc][:, tch * IC:(tch + 1) * IC],
                                    start=(dc == 0), stop=(dc == NDC - 1))
                            for hh in range(2):
                                l = 2 * fb + hh
                                dst = dest[l][0:64, tch * IC:(tch + 1) * IC]
                                if scl is None:
                                    nc.vector.tensor_copy(
                                        dst, pp[hh * D:(hh + 1) * D, :])
                                else:
                                    nc.vector.tensor_scalar_mul(
                                        dst, pp[hh * D:(hh + 1) * D, :], scl)
                    for tt4 in range(4):
                        tt = tch * 4 + tt4
                        pp = psA.tile([PB, FPC], F32, tag="pp", name=f"v{tt}")
                        for dc in range(NDC):
                            nc.tensor.matmul(
                                pp[:, :],
                                xT[dc][:, tt * PB:(tt + 1) * PB],
                                wT[dc][:, 2 * FPC:3 * FPC],
                                start=(dc == 0), stop=(dc == NDC - 1))
                        for l in range(HPC):
                            nc.vector.tensor_copy(vt[l][tt][:, 0:D],
                                                  pp[:, l * D:(l + 1) * D])
                            nc.vector.memset(vt[l][tt][:, D:D + 1], 1.0)

            proj(0)
            proj(1)

            # ---- attention + partial out-proj + RS, per i-chunk ----
            pending_rs = []       # staged, not yet triggered
            triggered_out = []    # triggered RSs awaiting out-copy

            def fire_rs(final=False):
                # out-copies first: their RS was triggered >= 1 chunk ago
                # so the wait doesn't stall the gpsimd queue. Then the
                # new triggers.
                for (t, k) in triggered_out:
                    for s in range(4):
                        nc.gpsimd.dma_start(
                            out=out_e[k * PB + s * 32:
                                      k * PB + (s + 1) * 32, :],
                            in_=t[s * 32:(s + 1) * 32, :])
                triggered_out.clear()
                for (args, kwargs, t, k) in pending_rs:
                    nc.gpsimd.collective_compute(*args, **kwargs)
                    triggered_out.append((t, k))
                pending_rs.clear()
                if final:
                    for (t, k) in triggered_out:
                        for s in range(4):
                            nc.gpsimd.dma_start(
                                out=out_e[k * PB + s * 32:
                                          k * PB + (s + 1) * 32, :],
                                in_=t[s * 32:(s + 1) * 32, :])
                    triggered_out.clear()

            for ichk in range(4):
                i0 = ichk * IC
                njt = i0 // JT + 4
                pvs = {}
                oTs = {}
                pend = []        # PV of pair n issues after QK of pair n+1

                def flush_pend():
                    for (l, jt, jstart, et, off, nn) in pend:
                        noff = IC - nn
                        nc.tensor.matmul(
                            pvs[l][0:D + 1, noff:IC],
                            vt[l][jt][:, :],
                            et[:, off:off + nn],
                            start=(jt == jstart), stop=(jt == njt - 1))
                    pend.clear()

                for l in range(HPC):
                    jstart = njt - min(njt, WTILES[l] + 4)
                    pvs[l] = psV.tile([D + 1, IC], F32, tag="pv",
                                      name=f"pv{l}_{ichk}")
                    with nc.named_scope(f"att{ichk}_{l}", notify=True):
                        for jp in range(jstart, njt, 2):
                            jts = list(range(jp, min(jp + 2, njt)))
                            spp = psS.tile([PB, 2 * IC], F32, tag="sp")
                            et = ep.tile([PB, 2 * IC], BF16, tag="ep")
                            nns = [IC - (max(i0, jt * JT) - i0)
                                   for jt in jts]
                            # pack tile B right after tile A when that
                            # stays inside PSUM bank 0 (one contiguous
                            # exp span instead of two)
                            offs = [0, nns[0]
                                    if nns[0] + (nns[1:] or [0])[0] <= IC
                                    else IC]
                            for h, jt in enumerate(jts):
                                j0 = jt * JT
                                ist = max(i0, j0)
                                nc.tensor.matmul(
                                    spp[:, offs[h]:offs[h] + nns[h]],
                                    kTt[l][:, j0:j0 + JT],
                                    qTt[l][:, ist:i0 + IC],
                                    start=True, stop=True)

                            if len(jts) == 2 and offs[1] == nns[0]:
                                nc.scalar.activation(
                                    et[:, 0:nns[0] + nns[1]],
                                    spp[:, 0:nns[0] + nns[1]],
                                    mybir.ActivationFunctionType.Exp,
                                    bias=nb8[:, :])
                            else:
                                for h, jt in enumerate(jts):
                                    nc.scalar.activation(
                                        et[:, offs[h]:offs[h] + nns[h]],
                                        spp[:, offs[h]:offs[h] + nns[h]],
                                        mybir.ActivationFunctionType.Exp,
                                        bias=nb8[:, :])
                            for h, jt in enumerate(jts):
                                if jt * JT >= i0:
                                    nc.gpsimd.tensor_tensor(
                                        et[:, offs[h]:offs[h] + JT],
                                        et[:, offs[h]:offs[h] + JT],
                                        mask[:, :], mybir.AluOpType.mult)
                            flush_pend()
                            for h, jt in enumerate(jts):
                                pend.append((l, jt, jstart, et, offs[h],
                                             nns[h]))
                    if l == 0:
                        # fire the previous chunk's RS here: its staging
                        # is done by now, and later gpsimd mask adds
                        # won't sit behind the trigger's semaphore wait.
                        fire_rs()
                    if l % 2 == 1:
                        flush_pend()
                        # normalize pair (l-1, l): one batched recip on
                        # partitions 0/32, DMA partition-broadcast,
                        # then per-head multiplies.
                        u = l // 2
                        dn = rcp.tile([33, IC], F32, tag="dn")
                        nc.vector.tensor_copy(dn[0:1, :],
                                              pvs[2 * u][D:D + 1, :])
                        nc.vector.tensor_copy(dn[32:33, :],
                                              pvs[2 * u + 1][D:D + 1, :])
                        rc = rcp.tile([33, IC], F32, tag="rc")
                        nc.vector.reciprocal_approx_fast(out=rc[:, :],
                                                         in_=dn[:, :])
                        bcs = bcp.tile([PB, IC], F32, tag="bcs")
                        nc.sync.dma_start(
                            out=bcs[0:D, :],
                            in_=rc[0:1, None, :].broadcast_to([1, D, IC]))
                        nc.sync.dma_start(
                            out=bcs[D:PB, :],
                            in_=rc[32:33, None, :].broadcast_to([1, D, IC]))
                        oT = op.tile([PB, IC], BF16, tag="oT",
                                     name=f"oT{u}_{ichk}")
                        nc.vector.tensor_tensor(
                            oT[0:D, :], pvs[2 * u][0:D, :], bcs[0:D, :],
                            mybir.AluOpType.mult)
                        nc.vector.tensor_tensor(
                            oT[D:PB, :], pvs[2 * u + 1][0:D, :], bcs[D:PB, :],
                            mybir.AluOpType.mult)
                        oTs[u] = oT

                # partial output projection for this 512-query window
                rs_in = rsi[ichk].tile([QUAD, PB, DM], BF16,
                                       tag=f"rsin{ichk}", name=f"rsi{ichk}")
                with nc.named_scope(f"oproj{ichk}", notify=True):
                    for tt4 in range(4):
                        ro = rop.tile([PB, DM], BF16, tag="ro",
                                      name=f"ro{ichk}_{tt4}")
                        for oc in range(2):
                            po = psA.tile([PB, IC], F32, tag="pp")
                            nc.tensor.matmul(
                                po[:, :],
                                oTs[0][:, tt4 * PB:(tt4 + 1) * PB],
                                woT[:, oc * IC:(oc + 1) * IC],
                                start=True, stop=False)
                            nc.tensor.matmul(
                                po[:, :],
                                oTs[1][:, tt4 * PB:(tt4 + 1) * PB],
                                woT[:, DM + oc * IC:DM + (oc + 1) * IC],
                                start=False, stop=True)
                            nc.vector.tensor_copy(
                                ro[:, oc * IC:(oc + 1) * IC], po[:, :])
                        nc.sync.dma_start(out=rs_in[tt4:tt4 + 1, :, :],
                                          in_=ro[:, :])
                rs_out = rso[ichk].tile([PB, DM], BF16,
                                        tag=f"rsout{ichk}",
                                        name=f"rso{ichk}")
                pending_rs.append((
                    ("ReduceScatter", mybir.AluOpType.add),
                    dict(replica_groups=[[0, 1, 2, 3], [4, 5, 6, 7]],
                         ins=[rs_in.opt()],
                         outs=[rs_out.opt()]),
                    rs_out, ichk))
                if ichk == 3:
                    fire_rs(final=True)

                if ichk + 2 < 4:
                    proj(ichk + 2)

    nc.compile()
    return nc


def _consts(m):
    """Per-core constant tensors; m = core % 4 (quad rank)."""
    bf = ml_dtypes.bfloat16

    def dec3(v):
        hi = v.astype(bf).astype(np.float32)
        mid = (v - hi).astype(bf).astype(np.float32)
        lo = (v - hi - mid).astype(bf).astype(np.float32)
        return hi, mid, lo

    heads = PERM0[m]
    slopes = [2.0 ** (-8.0 * (g + 1) / H) for g in heads]
    pos = np.arange(T, dtype=np.float32)
    kaug = np.zeros((6 * HPC, T), np.float32)
    qaug = np.zeros((6 * HPC, T), np.float32)
    for l, s in enumerate(slopes):
        kaug[6 * l:6 * l + 3] = dec3(s * pos)    # slope * j, 3-term exact
        kaug[6 * l + 3:6 * l + 6] = 1.0
        qaug[6 * l:6 * l + 3] = 1.0
        qaug[6 * l + 3:6 * l + 6] = dec3(-s * pos)  # -slope * i
    mask = (np.arange(PB)[None, :] >= np.arange(PB)[:, None]
            ).astype(bf)  # mask[jp, c]: c >= jp valid (multiplicative)
    return dict(mask=mask, kaug=kaug.astype(bf), qaug=qaug.astype(bf))


def _in_maps(x, Wq, Wk, Wv, Wo):
    bf = ml_dtypes.bfloat16
    x = np.asarray(x, np.float32)
    WqT = np.asarray(Wq, np.float32).T.astype(bf)   # (DM in, DM features)
    WkT = np.asarray(Wk, np.float32).T.astype(bf)
    WvT = np.asarray(Wv, np.float32).T.astype(bf)
    WoT = np.asarray(Wo, np.float32).T.astype(bf)   # (DM f, DM o)
    xw_b = []
    for b in range(B):
        xT = np.ascontiguousarray(x[b].T).astype(bf)     # (DM, T)
        xw_b.append(np.concatenate(
            [xT[dc * PB:(dc + 1) * PB, :] for dc in range(NDC)], axis=1))
    maps = []
    for c in range(NCORES):
        b, m = c // QUAD, c % QUAD
        cols = np.concatenate([np.arange(h * D, (h + 1) * D)
                               for h in PERM0[m]])
        wqkv = np.concatenate(
            [np.concatenate([WqT[dc * PB:(dc + 1) * PB][:, cols],
                             WkT[dc * PB:(dc + 1) * PB][:, cols],
                             WvT[dc * PB:(dc + 1) * PB][:, cols]], axis=1)
             for dc in range(NDC)], axis=1)
        wosel = WoT[cols, :]                       # (256 f, DM)
        wo = np.concatenate([wosel[0:PB, :], wosel[PB:2 * PB, :]], axis=1)
        mp = dict(xw=np.ascontiguousarray(xw_b[b]),
                  wqkv=np.ascontiguousarray(wqkv),
                  wo=np.ascontiguousarray(wo), **_consts(m))
        maps.append(mp)
    return maps


def _assemble(results):
    out = np.zeros((B, T, DM), np.float32)
    for c in range(NCORES):
        b, m = c // QUAD, c % QUAD
        core_out = np.asarray(results[c]["out"], np.float32)
        for k in range(4):
            tt = 4 * k + m
            out[b, tt * PB:(tt + 1) * PB, :] = core_out[k * PB:(k + 1) * PB, :]
    return out


def get_nc():
    if "nc" not in _cache:
        _cache["nc"] = _build()
    return _cache["nc"]


def run(inputs, trace=False, **kw):
    nc = get_nc()
    maps = _in_maps(**inputs)
    res = run_bass_kernel_spmd(nc, maps, core_ids=list(range(NCORES)),
                               trace=trace, **kw)
    return _assemble(res.results), res


def kernel(x, Wq, Wk, Wv, Wo):
    out, _ = run(dict(x=x, Wq=Wq, Wk=Wk, Wv=Wv, Wo=Wo))
    return out


# revision 15
# speedup vs baseline: 1.0968x; 1.0968x over previous
"""ALiBi causal attention on 8 TRN2 NeuronCores.

Sharding: core c handles batch b = c//4 and 4 heads (slope-grouped
permutation PERM0[c%4]); attention is fully local per core. The output
projection is computed as per-core PARTIAL products (own 256 features x
WoT rows) chunked by 512-query windows; a per-window 4-way ReduceScatter
(replica groups = batch quads) sums the partials and scatters each core
its own 128-row tile, writing straight into the bf16 output tensor
(host casts to f32). All but the last ReduceScatter hide under
attention compute (A2A has ~22-26us fixed cost; RS scales with bytes).

Score matmul trick: scores^T[j,i] = (q/8 . k)[j,i] + slope*j - slope*i
in one K=70 matmul; kT/qT rows 64-69 carry 3-term bf16 decompositions
of slope*j and -slope*i paired with ones. V gets a ones column so PV
also emits the softmax denominator (row 64).

Engine assignment (balance measured on traces):
- tensor: matmuls only.
- scalar (ACT): exp only (~59us) + early input-DMA issue.
- vector (DVE): all psum->sbuf copies, softmax normalize, memsets.
- gpsimd: warmup barrier, causal-mask adds, collective triggers.
- sync: staging/broadcast DMA issue.

Other structure:
- a zero-byte warmup AllGather fires at t~0 so core-start skew is
  absorbed on the CC engine concurrently with the fill phase instead of
  inside the first ReduceScatter.
- QK of pair n+1 issues before PV of pair n so exp hides under TensorE.
- RS_k's trigger is issued mid ichk k+1 so its semaphore wait doesn't
  block later gpsimd work (mask adds) while staging drains.
- reciprocals batched per head-pair on partitions 0/32 of one tile.
- ALiBi windows: dropped keys have slope*dist >= ~22 (weight < e^-18
  of max even with +-3 score noise).
"""

import sys

import numpy as np

try:
    import concourse  # noqa: F401
except ImportError:  # pragma: no cover
    sys.path.insert(0, "/opt/trn_rl_repo")

import ml_dtypes
from concourse import bacc, mybir
import concourse.tile as tile
from concourse.bass_utils import run_bass_kernel_spmd

BF16 = mybir.dt.bfloat16
F32 = mybir.dt.float32

B, T, DM, H = 2, 2048, 1024, 16
D = DM // H            # 64 head dim
NCORES = 8
QUAD = 4               # cores per batch
HPC = 4                # heads per core
PB = 128               # partitions
IC = 512               # i-chunk (query cols per window)
JT = 128               # j-tile (key rows per score tile)
NTT = T // PB          # 16 token tiles
NDC = DM // PB         # 8 d_model chunks
FPC = HPC * D          # 256 features per core
TOUT = T // QUAD       # 512 output rows per core
NEG = -1.0e9

import os as _os
PSA = int(_os.environ.get("PSA", 2))
PSS = int(_os.environ.get("PSS", 2))   # score tiles are 2 banks each
PSV = int(_os.environ.get("PSV", 2))
EPB = int(_os.environ.get("EPB", 4))

WTILES = [int(v) for v in _os.environ.get("WT", "1,3,11,16").split(",")]
PERM0 = [[m, 4 + m, 11 - m, 15 - m] for m in range(QUAD)]

_cache = {}


def _build():
    nc = bacc.Bacc("TRN2", target_bir_lowering=False, debug=False,
                   num_devices=NCORES)

    xw_e = nc.dram_tensor("xw", [PB, NDC * T], BF16, kind="ExternalInput")
    wqkv_e = nc.dram_tensor("wqkv", [PB, NDC * 3 * FPC], BF16,
                            kind="ExternalInput")
    wo_e = nc.dram_tensor("wo", [PB, 2 * DM], BF16, kind="ExternalInput")
    mask_e = nc.dram_tensor("mask", [PB, PB], F32, kind="ExternalInput")
    kaug_e = nc.dram_tensor("kaug", [6 * HPC, T], BF16, kind="ExternalInput")
    qaug_e = nc.dram_tensor("qaug", [6 * HPC, T], BF16, kind="ExternalInput")
    out_e = nc.dram_tensor("out", [TOUT, DM], BF16, kind="ExternalOutput")

    from contextlib import ExitStack
    with tile.TileContext(nc) as tc, ExitStack() as es:
            def pool(**kw):
                return es.enter_context(tc.tile_pool(**kw))
            xtp = pool(name="xt", bufs=NDC)        # xT chunks
            wtp = pool(name="wt", bufs=NDC)        # wqkv chunks
            wop = pool(name="wo", bufs=1)          # WoT own heads
            qkp = pool(name="qk", bufs=2 * HPC)    # qT/kT (70,T)
            vp = pool(name="vp", bufs=4 * NTT)     # v tiles (128,65)
            smp = pool(name="small", bufs=1)       # mask
            rcp = pool(name="rcp", bufs=4)         # recip rows
            bcp = pool(name="bcp", bufs=2)         # broadcast recip
            ep = pool(name="ep", bufs=EPB)         # exp tiles
            op = pool(name="op", bufs=4)           # oT tiles (2 per ichk)
            rop = pool(name="ro", bufs=4)          # staged bf16 proj rows
            psA = pool(name="psA", bufs=PSA, space="PSUM")  # proj/outproj
            psS = pool(name="psS", bufs=PSS, space="PSUM")  # score (2 bank)
            psV = pool(name="psV", bufs=PSV, space="PSUM")  # pv
            rsi = [pool(name=f"rsi{k}", bufs=1, space="DRAM")
                   for k in range(4)]
            rso = [pool(name=f"rso{k}", bufs=1, space="DRAM")
                   for k in range(4)]
            wup = pool(name="wup", bufs=1, space="DRAM")

            # ---- warmup barrier: absorb core-start skew on the CC
            # engine while the fill phase runs. Input is never written
            # (garbage data, pure barrier), output never read.
            wu_in = wup.tile([1, PB], BF16, tag="wui")
            wu_out = wup.tile([NCORES, 1, PB], BF16, tag="wuo")
            nc.gpsimd.collective_compute(
                "AllGather", mybir.AluOpType.bypass,
                replica_groups=[list(range(NCORES))],
                ins=[wu_in.opt()], outs=[wu_out.opt()])

            # ---- constants ----
            mask = smp.tile([PB, PB], F32, tag="mask")
            nc.sync.dma_start(out=mask[:, :], in_=mask_e[:, :])

            # ---- input DMAs. w chunks first (q/k proj contracts all of
            # them), then x token-halves in need order. DMA transfers
            # serialize through the 16-engine pool, so order == arrival.
            xT = [xtp.tile([PB, T], BF16, tag="xt", name=f"xT{dc}")
                  for dc in range(NDC)]
            wT = [wtp.tile([PB, 3 * FPC], BF16, tag="wt", name=f"wT{dc}")
                  for dc in range(NDC)]
            for dc in range(NDC):
                eng = nc.scalar if dc % 2 == 0 else nc.sync
                eng.dma_start(out=wT[dc][:, :],
                              in_=wqkv_e[:, dc * 3 * FPC:(dc + 1) * 3 * FPC])
            for dc in range(NDC):
                eng = nc.scalar if dc % 2 == 1 else nc.sync
                eng.dma_start(out=xT[dc][:, 0:T // 2],
                              in_=xw_e[:, dc * T:dc * T + T // 2])
            woT = wop.tile([PB, 2 * DM], BF16, tag="wo")
            nc.scalar.dma_start(out=woT[:, :], in_=wo_e[:, :])
            for dc in range(NDC):
                eng = nc.scalar if dc % 2 == 0 else nc.sync
                eng.dma_start(out=xT[dc][:, T // 2:T],
                              in_=xw_e[:, dc * T + T // 2:(dc + 1) * T])

            qTt = [qkp.tile([70, T], BF16, tag="qk", name=f"qT{l}")
                   for l in range(HPC)]
            kTt = [qkp.tile([70, T], BF16, tag="qk", name=f"kT{l}")
                   for l in range(HPC)]
            for l in range(HPC):
                nc.sync.dma_start(out=kTt[l][64:70, :],
                                  in_=kaug_e[6 * l:6 * l + 6, :])
                nc.sync.dma_start(out=qTt[l][64:70, :],
                                  in_=qaug_e[6 * l:6 * l + 6, :])

            vt4 = [vp.tile([PB, HPC * (D + 1)], BF16, tag="vp",
                           name=f"v4_{tt}")
                   for tt in range(NTT)]

            # ---- projections for one 512-token chunk (all 4 heads) ----
            def proj(tch):
                with nc.named_scope(f"proj{tch}", notify=True):
                    for wi, dest, scl in ((0, qTt, 0.125), (1, kTt, None)):
                        for fb in range(2):
                            pp = psA.tile([PB, IC], F32, tag="pp",
                                          name=f"qk{wi}{fb}{tch}")
                            for dc in range(NDC):
                                nc.tensor.matmul(
                                    pp[:, :],
                                    wT[dc][:, wi * FPC + fb * PB:
                                           wi * FPC + (fb + 1) * PB],
                                    xT[dc][:, tch * IC:(tch + 1) * IC],
                                    start=(dc == 0), stop=(dc == NDC - 1))
                            for hh in range(2):
                                l = 2 * fb + hh
                                dst = dest[l][0:64, tch * IC:(tch + 1) * IC]
                                if scl is None:
                                    if hh == 0:
                                        nc.scalar.copy(
                                            dst, pp[hh * D:(hh + 1) * D, :])
                                    else:
                                        nc.vector.tensor_copy(
                                            dst, pp[hh * D:(hh + 1) * D, :])
                                elif hh == 0:
                                    nc.scalar.mul(
                                        dst, pp[hh * D:(hh + 1) * D, :], scl)
                                else:
                                    nc.vector.tensor_scalar_mul(
                                        dst, pp[hh * D:(hh + 1) * D, :], scl)
                    for tt4 in range(4):
                        tt = tch * 4 + tt4
                        pp = psA.tile([PB, FPC], F32, tag="pp", name=f"v{tt}")
                        for dc in range(NDC):
                            nc.tensor.matmul(
                                pp[:, :],
                                xT[dc][:, tt * PB:(tt + 1) * PB],
                                wT[dc][:, 2 * FPC:3 * FPC],
                                start=(dc == 0), stop=(dc == NDC - 1))
                        vv = vt4[tt][:, :].rearrange(
                            "p (h c) -> p h c", h=HPC)
                        nc.vector.tensor_copy(
                            vv[:, :, 0:D],
                            pp[:, :].rearrange("p (h c) -> p h c", h=HPC))
                        nc.vector.memset(vv[:, :, D:D + 1], 1.0)

            proj(0)
            proj(1)

            # ---- attention + partial out-proj + RS, per i-chunk ----
            pending_rs = []       # staged, not yet triggered
            triggered_out = []    # triggered RSs awaiting out-copy

            def fire_rs(final=False):
                # out-copies first: their RS was triggered >= 1 chunk ago
                # so the wait doesn't stall the gpsimd queue. Then the
                # new triggers.
                for (t, k) in triggered_out:
                    for s in range(4):
                        nc.gpsimd.dma_start(
                            out=out_e[k * PB + s * 32:
                                      k * PB + (s + 1) * 32, :],
                            in_=t[s * 32:(s + 1) * 32, :])
                triggered_out.clear()
                for (args, kwargs, t, k) in pending_rs:
                    nc.gpsimd.collective_compute(*args, **kwargs)
                    triggered_out.append((t, k))
                pending_rs.clear()
                if final:
                    for (t, k) in triggered_out:
                        for s in range(4):
                            nc.gpsimd.dma_start(
                                out=out_e[k * PB + s * 32:
                                          k * PB + (s + 1) * 32, :],
                                in_=t[s * 32:(s + 1) * 32, :])
                    triggered_out.clear()

            for ichk in range(4):
                i0 = ichk * IC
                njt = i0 // JT + 4
                pvs = {}
                oTs = {}
                pend = []        # PV of pair n issues after QK of pair n+1

                def flush_pend():
                    for (l, jt, jstart, et, off, nn) in pend:
                        noff = IC - nn
                        nc.tensor.matmul(
                            pvs[l][0:D + 1, noff:IC],
                            vt4[jt][:, l * (D + 1):(l + 1) * (D + 1)],
                            et[:, off:off + nn],
                            start=(jt == jstart), stop=(jt == njt - 1))
                    pend.clear()

                for l in range(HPC):
                    jstart = njt - min(njt, WTILES[l] + 4)
                    pvs[l] = psV.tile([D + 1, IC], F32, tag="pv",
                                      name=f"pv{l}_{ichk}")
                    with nc.named_scope(f"att{ichk}_{l}", notify=True):
                        for jp in range(jstart, njt, 2):
                            jts = list(range(jp, min(jp + 2, njt)))
                            spp = psS.tile([PB, 2 * IC], F32, tag="sp")
                            et = ep.tile([PB, 2 * IC], BF16, tag="ep")
                            nns = [IC - (max(i0, jt * JT) - i0)
                                   for jt in jts]
                            # pack tile B right after tile A when that
                            # stays inside PSUM bank 0 (one contiguous
                            # exp span instead of two)
                            offs = [0, nns[0]
                                    if nns[0] + (nns[1:] or [0])[0] <= IC
                                    else IC]
                            for h, jt in enumerate(jts):
                                j0 = jt * JT
                                ist = max(i0, j0)
                                nc.tensor.matmul(
                                    spp[:, offs[h]:offs[h] + nns[h]],
                                    kTt[l][:, j0:j0 + JT],
                                    qTt[l][:, ist:i0 + IC],
                                    start=True, stop=True)
                                if j0 >= i0:
                                    nc.vector.tensor_add(
                                        spp[:, offs[h]:offs[h] + JT],
                                        spp[:, offs[h]:offs[h] + JT],
                                        mask[:, :])

                            if len(jts) == 2 and offs[1] == nns[0]:
                                nc.scalar.activation(
                                    et[:, 0:nns[0] + nns[1]],
                                    spp[:, 0:nns[0] + nns[1]],
                                    mybir.ActivationFunctionType.Exp)
                            else:
                                for h, jt in enumerate(jts):
                                    nc.scalar.activation(
                                        et[:, offs[h]:offs[h] + nns[h]],
                                        spp[:, offs[h]:offs[h] + nns[h]],
                                        mybir.ActivationFunctionType.Exp)
                            flush_pend()
                            for h, jt in enumerate(jts):
                                pend.append((l, jt, jstart, et, offs[h],
                                             nns[h]))
                    if l == 0:
                        # fire the previous chunk's RS here: its staging
                        # is done by now, and later gpsimd mask adds
                        # won't sit behind the trigger's semaphore wait.
                        fire_rs()
                    if l % 2 == 1:
                        flush_pend()
                        # normalize pair (l-1, l): one batched recip on
                        # partitions 0/32, DMA partition-broadcast,
                        # then per-head multiplies.
                        u = l // 2
                        dn = rcp.tile([33, IC], F32, tag="dn")
                        nc.vector.tensor_copy(dn[0:1, :],
                                              pvs[2 * u][D:D + 1, :])
                        nc.vector.tensor_copy(dn[32:33, :],
                                              pvs[2 * u + 1][D:D + 1, :])
                        rc = rcp.tile([33, IC], F32, tag="rc")
                        nc.vector.reciprocal_approx_fast(out=rc[:, :],
                                                         in_=dn[:, :])
                        bcs = bcp.tile([PB, IC], F32, tag="bcs")
                        nc.sync.dma_start(
                            out=bcs[0:D, :],
                            in_=rc[0:1, None, :].broadcast_to([1, D, IC]))
                        nc.sync.dma_start(
                            out=bcs[D:PB, :],
                            in_=rc[32:33, None, :].broadcast_to([1, D, IC]))
                        oT = op.tile([PB, IC], BF16, tag="oT",
                                     name=f"oT{u}_{ichk}")
                        nc.vector.tensor_tensor(
                            oT[0:D, :], pvs[2 * u][0:D, :], bcs[0:D, :],
                            mybir.AluOpType.mult)
                        nc.vector.tensor_tensor(
                            oT[D:PB, :], pvs[2 * u + 1][0:D, :], bcs[D:PB, :],
                            mybir.AluOpType.mult)
                        oTs[u] = oT

                # partial output projection for this 512-query window
                rs_in = rsi[ichk].tile([QUAD, PB, DM], BF16,
                                       tag=f"rsin{ichk}", name=f"rsi{ichk}")
                with nc.named_scope(f"oproj{ichk}", notify=True):
                    for tt4 in range(4):
                        ro = rop.tile([PB, DM], BF16, tag="ro",
                                      name=f"ro{ichk}_{tt4}")
                        for oc in range(2):
                            po = psA.tile([PB, IC], F32, tag="pp")
                            nc.tensor.matmul(
                                po[:, :],
                                oTs[0][:, tt4 * PB:(tt4 + 1) * PB],
                                woT[:, oc * IC:(oc + 1) * IC],
                                start=True, stop=False)
                            nc.tensor.matmul(
                                po[:, :],
                                oTs[1][:, tt4 * PB:(tt4 + 1) * PB],
                                woT[:, DM + oc * IC:DM + (oc + 1) * IC],
                                start=False, stop=True)
                            if oc == 0:
                                nc.scalar.copy(ro[:, 0:IC], po[:, :])
                            else:
                                nc.vector.tensor_copy(ro[:, IC:DM],
                                                      po[:, :])
                        nc.sync.dma_start(out=rs_in[tt4:tt4 + 1, :, :],
                                          in_=ro[:, :])
                rs_out = rso[ichk].tile([PB, DM], BF16,
                                        tag=f"rsout{ichk}",
                                        name=f"rso{ichk}")
                pending_rs.append((
                    ("ReduceScatter", mybir.AluOpType.add),
                    dict(replica_groups=[[0, 1, 2, 3], [4, 5, 6, 7]],
                         ins=[rs_in.opt()],
                         outs=[rs_out.opt()]),
                    rs_out, ichk))
                if ichk == 3:
                    fire_rs(final=True)

                if ichk + 2 < 4:
                    proj(ichk + 2)

    nc.compile()
    return nc


def _consts(m):
    """Per-core constant tensors; m = core % 4 (quad rank)."""
    bf = ml_dtypes.bfloat16

    def dec3(v):
        hi = v.astype(bf).astype(np.float32)
        mid = (v - hi).astype(bf).astype(np.float32)
        lo = (v - hi - mid).astype(bf).astype(np.float32)
        return hi, mid, lo

    heads = PERM0[m]
    slopes = [2.0 ** (-8.0 * (g + 1) / H) for g in heads]
    pos = np.arange(T, dtype=np.float32)
    kaug = np.zeros((6 * HPC, T), np.float32)
    qaug = np.zeros((6 * HPC, T), np.float32)
    for l, s in enumerate(slopes):
        kaug[6 * l:6 * l + 3] = dec3(s * pos)    # slope * j, 3-term exact
        kaug[6 * l + 3:6 * l + 6] = 1.0
        qaug[6 * l:6 * l + 3] = 1.0
        qaug[6 * l + 3:6 * l + 6] = dec3(-s * pos)  # -slope * i
    mask = np.where(np.arange(PB)[None, :] >= np.arange(PB)[:, None],
                    0.0, NEG).astype(np.float32)  # mask[jp, c]: c >= jp valid
    return dict(mask=mask, kaug=kaug.astype(bf), qaug=qaug.astype(bf))


def _in_maps(x, Wq, Wk, Wv, Wo):
    bf = ml_dtypes.bfloat16
    x = np.asarray(x, np.float32)
    WqT = np.asarray(Wq, np.float32).T.astype(bf)   # (DM in, DM features)
    WkT = np.asarray(Wk, np.float32).T.astype(bf)
    WvT = np.asarray(Wv, np.float32).T.astype(bf)
    WoT = np.asarray(Wo, np.float32).T.astype(bf)   # (DM f, DM o)
    xw_b = []
    for b in range(B):
        xT = np.ascontiguousarray(x[b].T).astype(bf)     # (DM, T)
        xw_b.append(np.concatenate(
            [xT[dc * PB:(dc + 1) * PB, :] for dc in range(NDC)], axis=1))
    maps = []
    for c in range(NCORES):
        b, m = c // QUAD, c % QUAD
        cols = np.concatenate([np.arange(h * D, (h + 1) * D)
                               for h in PERM0[m]])
        wqkv = np.concatenate(
            [np.concatenate([WqT[dc * PB:(dc + 1) * PB][:, cols],
                             WkT[dc * PB:(dc + 1) * PB][:, cols],
                             WvT[dc * PB:(dc + 1) * PB][:, cols]], axis=1)
             for dc in range(NDC)], axis=1)
        wosel = WoT[cols, :]                       # (256 f, DM)
        wo = np.concatenate([wosel[0:PB, :], wosel[PB:2 * PB, :]], axis=1)
        mp = dict(xw=np.ascontiguousarray(xw_b[b]),
                  wqkv=np.ascontiguousarray(wqkv),
                  wo=np.ascontiguousarray(wo), **_consts(m))
        maps.append(mp)
    return maps


def _assemble(results):
    out = np.zeros((B, T, DM), np.float32)
    for c in range(NCORES):
        b, m = c // QUAD, c % QUAD
        core_out = np.asarray(results[c]["out"], np.float32)
        for k in range(4):
            tt = 4 * k + m
            out[b, tt * PB:(tt + 1) * PB, :] = core_out[k * PB:(k + 1) * PB, :]
    return out


def get_nc():
    if "nc" not in _cache:
        _cache["nc"] = _build()
    return _cache["nc"]


def run(inputs, trace=False, **kw):
    nc = get_nc()
    maps = _in_maps(**inputs)
    res = run_bass_kernel_spmd(nc, maps, core_ids=list(range(NCORES)),
                               trace=trace, **kw)
    return _assemble(res.results), res


def kernel(x, Wq, Wk, Wv, Wo):
    out, _ = run(dict(x=x, Wq=Wq, Wk=Wk, Wv=Wv, Wo=Wo))
    return out
